# revision 19
# baseline (speedup 1.0000x reference)
"""Trainium2 Bass kernel for the dynamic mixture-of-operators routing module.

Reference computation (per image):
  g = GAP(x) -> 1x1 conv -> [4]            (global router)
  s = 1x1 conv(x) -> [4, H, W]             (spatial router)
  logits = g + s;  weights = softmax(logits, op axis)
  per-pixel top-2 mask + renormalize (+eps)
  out = x*w0 + conv3x3(x)*w1 + conv5x5(x)*w2 + avgpool3x3(x)*w3

Strategy: data-parallel over batch (16 images -> 8 cores x 2 images).
Convs computed as matmuls over channel chunks with PSUM accumulation over
the kernel taps; per-pixel routing math on pixel-partitioned [128, 32]
tiles; mix consumed straight from PSUM per 8-row group.
"""

import os
import sys

_TRN = "/opt/trn_rl_repo"
if _TRN not in sys.path:
    sys.path.insert(0, _TRN)

import numpy as np

import concourse.bass as bass
import concourse.tile as tile
from concourse import mybir
from concourse.alu_op_type import AluOpType as Op
from bass_rust import ActivationFunctionType as Act, AxisListType

# ---------------------------------------------------------------------------
# The walrus shipped in this container rejects instructions carrying more
# than one sync wait (setupSyncWait: "Too many sync wait commands").  The
# stock TileContext tail drain waits on every live semaphore at once; split
# it into one drain per waited proc.
# ---------------------------------------------------------------------------
import re as _re
from concourse.vector_clock import ScopedClock as _ScopedClock, VectorClock as _VectorClock


def _split_drain_and_barrier(self, tick_clock, wait_clock):
    nc = self.nc
    gc = tick_clock.global_clock
    vals = [int(s) for s in _re.findall(r"\d+", repr(gc))]
    nonzero = [i for i, v in enumerate(vals) if v != 0]
    if not nonzero:
        d = nc.sync.drain()
        wait_clock.add_sem_waits(d.ins, _ScopedClock({None: gc}))
    for i in nonzero:
        sub = [0] * len(vals)
        sub[i] = vals[i]
        d = nc.sync.drain()
        wait_clock.add_sem_waits(d.ins, _ScopedClock({None: _VectorClock(sub)}))

    nc.all_engine_barrier()
    assert self.sems is not None
    popped = nc._tile_sem_poison_stack.pop()
    assert popped is self._sem_poison
    nc.clear_and_free_semaphores(list(self.sems.allocated().values()))
    nc.all_engine_barrier()


tile.TileContext._drain_and_barrier = _split_drain_and_barrier

# ---------------------------------------------------------------------------
# Problem constants (hardcoded per the task contract).
# ---------------------------------------------------------------------------
B, C, H, W = 16, 256, 64, 64
OPS = 4
EPS = 1e-6
NCORES = 8
BPC = B // NCORES          # images per core
P = 128                    # partitions
CK = C // P                # channel chunks
NPIX = H * W               # 4096
PH = PW = 68               # padded image (pad=2 each side)
NRG = 8                    # row groups
RGR = H // NRG             # rows per group (8)
RGPIX = RGR * W            # pixels per group (512)
BIG = 1.0e30

F32 = mybir.dt.float32
F32R = mybir.dt.float32r
BF16 = mybir.dt.bfloat16


def _emit(nc, tc, ctx, io):
    x, out = io["x"], io["out"]
    gr_w, gr_b, sr_w, sr_b = io["gr_w"], io["gr_b"], io["sr_w"], io["sr_b"]
    conv3_w, conv3_b, conv5_w, conv5_b = (
        io["conv3_w"], io["conv3_b"], io["conv5_w"], io["conv5_b"])

    sync = nc.sync

    stg = ctx.enter_context(tc.tile_pool(name="stg", bufs=2))
    wpool = ctx.enter_context(tc.tile_pool(name="wpool", bufs=1))
    xpool = ctx.enter_context(tc.tile_pool(name="xpool", bufs=1))
    smalls = ctx.enter_context(tc.tile_pool(name="smalls", bufs=1))
    rt = ctx.enter_context(tc.tile_pool(name="rt", bufs=2))
    wb = ctx.enter_context(tc.tile_pool(name="wb", bufs=2))
    mixp = ctx.enter_context(tc.tile_pool(name="mixp", bufs=2))
    accp = ctx.enter_context(tc.tile_pool(name="accp", bufs=3))
    psum_r = ctx.enter_context(tc.tile_pool(name="psum_r", bufs=2, space="PSUM"))
    psum_c = ctx.enter_context(tc.tile_pool(name="psum_c", bufs=6, space="PSUM"))
    dram = ctx.enter_context(tc.tile_pool(name="dram", bufs=1, space="DRAM"))

    # --- conv weights: [co, ci, ky, kx] f32 -> per-ci-chunk bf16 [128, 256co, KK] ---
    wsb = {}
    for kk, w_dram in ((3, conv3_w), (5, conv5_w)):
        nt = kk * kk
        wsb[kk] = [wpool.tile([P, C, nt], BF16, tag=f"w{kk}sb{ci}",
                              name=f"w{kk}sb{ci}") for ci in range(CK)]
        for ci in range(CK):
            if kk == 3:
                halves = [(0, C)]
            else:
                halves = [(0, P), (P, C)]
            for c0, c1 in halves:
                ncol = c1 - c0
                stgt = stg.tile([P, ncol, nt], F32, tag="sx")
                src = (w_dram[c0:c1, ci * P:(ci + 1) * P, :, :]
                       .rearrange("co p kh kw -> p co (kh kw)"))
                sync.dma_start(out=stgt, in_=src)
                nc.vector.tensor_copy(wsb[kk][ci][:, c0:c1, :], stgt)

    # --- router weights [o, c] -> [c, o] per chunk; biases ---
    srwT = smalls.tile([P, CK, OPS], F32)
    grwT = smalls.tile([P, CK, OPS], F32)
    for wt, wd in ((srwT, sr_w), (grwT, gr_w)):
        src = wd.rearrange("o (k p) kh kw -> p k (o kh kw)", p=P)
        for k in range(CK):
            sync.dma_start(out=wt[:, k, :], in_=src[:, k, :])
    srb = smalls.tile([OPS, 1], F32)
    grb = smalls.tile([OPS, 1], F32)
    sync.dma_start(out=srb, in_=sr_b.unsqueeze(1))
    sync.dma_start(out=grb, in_=gr_b.unsqueeze(1))
    bsum = smalls.tile([OPS, 1], F32)
    nc.vector.tensor_add(bsum, srb, grb)

    b3sb = smalls.tile([P, CK], F32)
    b5sb = smalls.tile([P, CK], F32)
    for bt, bd in ((b3sb, conv3_b), (b5sb, conv5_b)):
        sync.dma_start(out=bt, in_=bd.rearrange("(k p) -> p k", p=P))

    # routing weight rows, written per image then broadcast-read per row group
    wrow = dram.tile([BPC, OPS, NPIX], F32)

    # persistent padded-x tiles (borders zeroed once)
    xpad = [xpool.tile([P, CK, PH * PW], BF16, tag=f"xpad{b}", name=f"xpad{b}")
            for b in range(BPC)]
    for t in xpad:
        nc.gpsimd.memset(t, 0.0)

    for b in range(BPC):
        xpv = [xpad[b][:, ci, :].rearrange("p (r c) -> p r c", c=PW) for ci in range(CK)]

        # ---- load x, build padded bf16 copy, row sums for global router ----
        sx = []
        xsum = []
        for ci in range(CK):
            s = stg.tile([P, NPIX], F32, tag="sx")
            sync.dma_start(out=s, in_=x[b, ci * P:(ci + 1) * P, :, :].rearrange("c h w -> c (h w)"))
            sx.append(s)
            xs = smalls.tile([P, 1], F32, tag=f"xsum{ci}")
            nc.vector.tensor_reduce(xs, s, axis=AxisListType.X, op=Op.add)
            xsum.append(xs)
            nc.vector.tensor_copy(
                xpv[ci][:, 2:2 + H, 2:2 + W],
                s.rearrange("p (h w) -> p h w", w=W),
            )

        # ---- global router: g = grwT.T @ mean(x) ----
        pg = psum_r.tile([OPS, 1], F32, tag="ps_s")
        for ci in range(CK):
            nc.tensor.matmul(pg, grwT[:, ci, :], xsum[ci],
                             start=(ci == 0), stop=(ci == CK - 1))
        g_sb = smalls.tile([OPS, 1], F32, tag="g_sb")
        # g/4096 + (gr_b + sr_b)
        nc.vector.scalar_tensor_tensor(g_sb, pg, 1.0 / NPIX, bsum, Op.mult, Op.add)

        # ---- spatial router + transpose to pixel-partitioned layout ----
        R = rt.tile([P, OPS, 32], F32, tag="R")
        for n in range(NPIX // 512):
            psn = psum_r.tile([OPS, 512], F32, tag="ps_s")
            for ci in range(CK):
                nc.tensor.matmul(psn, srwT[:, ci, :],
                                 sx[ci][:, n * 512:(n + 1) * 512],
                                 start=(ci == 0), stop=(ci == CK - 1))
            s2 = smalls.tile([OPS, 512], F32, tag="s2")
            nc.vector.tensor_scalar(s2, psn, g_sb, None, op0=Op.add)
            for o in range(OPS):
                sync.dma_start(out=R[16 * n:16 * (n + 1), o, :], in_=s2[o:o + 1, :])

        # ---- per-pixel softmax + top-2 mask + renorm ----
        M1 = rt.tile([P, 32], F32, tag="M1")
        nc.vector.tensor_tensor(M1, R[:, 0, :], R[:, 1, :], Op.max)
        nc.vector.tensor_tensor(M1, M1, R[:, 2, :], Op.max)
        nc.vector.tensor_tensor(M1, M1, R[:, 3, :], Op.max)
        E = rt.tile([P, OPS, 32], F32, tag="E")
        for o in range(OPS):
            nc.vector.tensor_sub(E[:, o, :], R[:, o, :], M1)
            nc.scalar.activation(E[:, o, :], E[:, o, :], Act.Exp)
        Z = rt.tile([P, 32], F32, tag="Z")
        nc.vector.tensor_add(Z, E[:, 0, :], E[:, 1, :])
        nc.vector.tensor_add(Z, Z, E[:, 2, :])
        nc.vector.tensor_add(Z, Z, E[:, 3, :])
        T = rt.tile([P, OPS, 32], F32, tag="T")
        for o in range(OPS):
            nc.vector.tensor_tensor(T[:, o, :], R[:, o, :], M1, Op.is_equal)
            nc.vector.scalar_tensor_tensor(T[:, o, :], T[:, o, :], -BIG, R[:, o, :],
                                           Op.mult, Op.add)
        M2 = rt.tile([P, 32], F32, tag="M2")
        nc.vector.tensor_tensor(M2, T[:, 0, :], T[:, 1, :], Op.max)
        nc.vector.tensor_tensor(M2, M2, T[:, 2, :], Op.max)
        nc.vector.tensor_tensor(M2, M2, T[:, 3, :], Op.max)
        for o in range(OPS):
            nc.vector.tensor_tensor(T[:, o, :], R[:, o, :], M2, Op.is_ge)
            nc.vector.tensor_mul(E[:, o, :], E[:, o, :], T[:, o, :])
        Ssum = rt.tile([P, 32], F32, tag="S")
        nc.vector.tensor_add(Ssum, E[:, 0, :], E[:, 1, :])
        nc.vector.tensor_add(Ssum, Ssum, E[:, 2, :])
        nc.vector.tensor_add(Ssum, Ssum, E[:, 3, :])
        den = rt.tile([P, 32], F32, tag="den")
        nc.vector.scalar_tensor_tensor(den, Z, EPS, Ssum, Op.mult, Op.add)
        nc.vector.reciprocal(den, den)
        for o in range(OPS):
            if o == 3:
                # fold the avgpool 1/9 into the routing weight
                nc.vector.scalar_tensor_tensor(E[:, o, :], E[:, o, :], 1.0 / 9.0, den,
                                               Op.mult, Op.mult)
            else:
                nc.vector.tensor_mul(E[:, o, :], E[:, o, :], den)
            sync.dma_start(out=wrow[b, o, :], in_=E[:, o, :])

        # ---- convs + mix, one 8-row group at a time ----
        for rg in range(NRG):
            y0 = rg * RGR
            Wb = wb.tile([P, OPS, RGR, W], F32, tag="Wb")
            for o in range(OPS):
                sync.dma_start(out=Wb[:, o], in_=wrow[b, o, rg * RGPIX:(rg + 1) * RGPIX]
                               .partition_broadcast(P))

            ps = {}
            for kk, off in ((3, 1), (5, 0)):
                nt = kk * kk
                for co in range(CK):
                    pc = psum_c.tile([P, RGR, W], F32, tag="pc")
                    ps[(kk, co)] = pc
                    for ci in range(CK):
                        for t in range(nt):
                            dy, dx = t // kk, t % kk
                            rhs = xpv[ci][:, y0 + dy + off: y0 + dy + off + RGR,
                                          dx + off: dx + off + W]
                            nc.tensor.matmul(pc, wsb[kk][ci][:, co * P:(co + 1) * P, t],
                                             rhs,
                                             start=(ci == 0 and t == 0),
                                             stop=(ci == CK - 1 and t == nt - 1))

            for co in range(CK):
                # 3x3 box sum of x (for avgpool; /9 folded into Wb[3])
                rs = mixp.tile([P, RGR + 2, W], F32, tag="rs")
                nc.vector.tensor_tensor(rs, xpv[co][:, y0 + 1: y0 + RGR + 3, 1:1 + W],
                                        xpv[co][:, y0 + 1: y0 + RGR + 3, 2:2 + W], Op.add)
                nc.vector.tensor_tensor(rs, rs,
                                        xpv[co][:, y0 + 1: y0 + RGR + 3, 3:3 + W], Op.add)
                p3 = mixp.tile([P, RGR, W], F32, tag="p3")
                nc.vector.tensor_tensor(p3, rs[:, 0:RGR, :], rs[:, 1:RGR + 1, :], Op.add)
                nc.vector.tensor_tensor(p3, p3, rs[:, 2:RGR + 2, :], Op.add)

                t1 = mixp.tile([P, RGR, W], F32, tag="t1")
                nc.vector.scalar_tensor_tensor(t1, ps[(3, co)], b3sb[:, co:co + 1],
                                               Wb[:, 1], Op.add, Op.mult)
                t2 = mixp.tile([P, RGR, W], F32, tag="t2")
                nc.vector.scalar_tensor_tensor(t2, ps[(5, co)], b5sb[:, co:co + 1],
                                               Wb[:, 2], Op.add, Op.mult)
                a = accp.tile([P, RGR, W], F32, tag="acc")
                nc.vector.tensor_tensor(a, xpv[co][:, y0 + 2: y0 + RGR + 2, 2:2 + W],
                                        Wb[:, 0], Op.mult)
                nc.vector.tensor_add(a, a, t1)
                nc.vector.tensor_add(a, a, t2)
                nc.vector.tensor_tensor(p3, p3, Wb[:, 3], Op.mult)
                nc.vector.tensor_add(a, a, p3)
                sync.dma_start(out=out[b, co * P:(co + 1) * P, y0:y0 + RGR, :], in_=a)


def build_module():
    from concourse import bacc
    nc = bacc.Bacc("TRN2")
    io = {
        "x": nc.dram_tensor("x", [BPC, C, H, W], F32, kind="ExternalInput").ap(),
        "gr_w": nc.dram_tensor("gr_w", [OPS, C, 1, 1], F32, kind="ExternalInput").ap(),
        "gr_b": nc.dram_tensor("gr_b", [OPS], F32, kind="ExternalInput").ap(),
        "sr_w": nc.dram_tensor("sr_w", [OPS, C, 1, 1], F32, kind="ExternalInput").ap(),
        "sr_b": nc.dram_tensor("sr_b", [OPS], F32, kind="ExternalInput").ap(),
        "conv3_w": nc.dram_tensor("conv3_w", [C, C, 3, 3], F32, kind="ExternalInput").ap(),
        "conv3_b": nc.dram_tensor("conv3_b", [C], F32, kind="ExternalInput").ap(),
        "conv5_w": nc.dram_tensor("conv5_w", [C, C, 5, 5], F32, kind="ExternalInput").ap(),
        "conv5_b": nc.dram_tensor("conv5_b", [C], F32, kind="ExternalInput").ap(),
        "out": nc.dram_tensor("out", [BPC, C, H, W], F32, kind="ExternalOutput").ap(),
    }
    from contextlib import ExitStack
    with tile.TileContext(nc) as tc:
        with ExitStack() as ctx:
            _emit(nc, tc, ctx, io)
    nc.compile()
    return nc


_LAST_RESULTS = None


def kernel(**inputs) -> np.ndarray:
    global _LAST_RESULTS
    arrs = {k: np.ascontiguousarray(np.asarray(v, dtype=np.float32))
            for k, v in inputs.items()}
    nc = build_module()

    shared = {k: v for k, v in arrs.items() if k != "x"}
    in_maps = []
    for i in range(NCORES):
        m = dict(shared)
        m["x"] = arrs["x"][i * BPC:(i + 1) * BPC]
        in_maps.append(m)

    from concourse.bass_utils import run_bass_kernel_spmd
    trace = bool(os.environ.get("BASS_KERNEL_TRACE"))
    try:
        res = run_bass_kernel_spmd(nc, in_maps, core_ids=list(range(NCORES)),
                                   trace=trace)
    except Exception:
        if not trace:
            raise
        res = run_bass_kernel_spmd(nc, in_maps, core_ids=list(range(NCORES)),
                                   trace=False)
    _LAST_RESULTS = res
    return np.concatenate([r["out"] for r in res.results], axis=0)
